# revision 1
# baseline (speedup 1.0000x reference)
"""Trainium2 Bass kernel for nn_CNN_Mem (CNN text encoder + cosine memory lookup).

Strategy (8 NeuronCores, SPMD):
  - Memory bank sharded along mem_size: host label-sorts mem_keys so every
    16-column block holds a single label (groups padded by duplicating a real
    key of the same label -> maxes are exact), casts to fp16, transposes to
    [300, M/8] slabs per core.
  - Each core: CNN for its 16 batch rows (embedding rows gathered host-side,
    convs as PSUM-accumulated matmuls over shifted APs, relu+bias on ACT,
    maxpool on DVE) -> feature chunks [100, 16] per kernel size. These are
    exactly the d-chunks of q^T. AllGather across the 8 cores -> lhsT
    [100, 128] per d-chunk.
  - Stream the keysT slab through the PE in [128, 512] PSUM chunks
    (3 accumulated fp16 matmuls each), segmented reduce_max (blocks of 16)
    -> per-block maxes; then one small masked-max pass over block labels
    gives per-core sim_pos/sim_neg partials (shifted by +SHIFT so empty
    positives read as 0).
  - Host combines: max over cores, divide by feature norms (maxes commute
    with the positive per-row normalization, so the kernel works on
    unnormalized features; norms come back via a sumsq output), then
    loss = mean(relu(sim_neg - sim_pos + margin)),
    accuracy = mean(sim_pos > sim_neg)  (equivalent to the argmax form for
    distinct maxima: the nearest neighbour's label matches y iff the best
    positive beats the best negative).

Performance notes:
  - fp16 keys halve the streamed bytes (dominant cost: 262144x300 bank ->
    ~20 MB/core) and run the PE at 1 cycle/row; fp32 margin analysis shows
    |sim_pos - sim_neg| >= 0.034 per row vs ~1e-4 fp16-induced error.
  - SBUF DMA ports are bound to partition octets (even SDMA engines serve
    partitions 0-63, odd 64-127), so a [100, N] transfer runs at ~78% of
    peak. Each d-chunk's 100 rows are therefore scattered over all 128
    partitions (3 stride-4 stripes + 4 extras on distinct ports ->
    ~98.7% port balance); the matching lhsT is built on-device with a
    permutation matmul whose zero rows also nullify the never-written
    partitions of the key tiles.
  - PSUM chunk maxes are copied to SBUF as f16 by the otherwise-idle ACT
    engine so the DVE segmented reduce runs in 2-4x perf mode; label
    masks (+-16 sentinels, exact in f16) are precomputed while the CNN
    runs, leaving a ~3 us masked-max tail.
  - Conv weights are padded to 128 output channels so Fast-Weight-Load
    engages on the CNN matmuls; the feature AllGather is issued before the
    sumsq block so collective latency overlaps remaining CNN work; the
    permutation matmuls run in f32 directly on the gathered features
    (no separate cast pass).
  - Correctness vs the f32 jax reference: rel err ~6e-5 on loss, accuracy
    exact. DMA-busy floor for the fp16 stream is ~59 us/core at balanced
    ports; cost-model (port/FWL-blind) says ~0.1-0.15 ms.
"""
import numpy as np
from contextlib import ExitStack

import concourse.bass as bass
import concourse.tile as tile
from concourse import bacc, mybir
from concourse.bass_utils import run_bass_kernel_spmd

# ---- problem dims (hardcoded; harness passes matching inputs) ----
B, L = 128, 64
V, D = 25000, 300
C = 1000
KN = 100
KSIZES = (3, 4, 5)
M, KEY = 262144, 300
MARGIN = 0.1

N_CORES = 8
BPC = B // N_CORES          # batch rows per core
TOK = BPC * L               # tokens per core
DCN = 3                     # d-chunks of 100
DCW = 100                   # d-chunk width
KNP = 128                   # conv output channels padded for FWL
CHUNK = 512                 # sim columns per PSUM chunk
BLK = 16                    # label-pure block width
NCH = 66                    # chunks per core
G = 6                       # chunks per DMA group
W = NCH * CHUNK             # slab columns per core (33792)
CAP = N_CORES * W           # padded memory size (270336)
NBLK = W // BLK             # blocks per core (2112)
BIG = 16.0                  # mask sentinel; |sims_u| <= ~8 so +-16 acts as +-inf

f32 = mybir.dt.float32
f16 = mybir.dt.float16

# port-balanced partition scatter: each d-chunk's 100 rows live on
# partitions {p : p%4 < 3} (96 stripe rows) plus 4 extras on distinct
# DMA ports per chunk; remaining rows are zeros in the stationary operand.
XTRA = (3, 7, 67)           # extras offset per d-chunk (step 8, count 4)


def _scatter_partitions(dc):
    ps = [4 * a + i for a in range(32) for i in range(3)]
    ps += [XTRA[dc] + 8 * j for j in range(4)]
    return ps

_CACHED_NC = None


def build(collective=True, g=11, ktbufs=4, skip_cnn=False, balanced=True):
    nc = bacc.Bacc("TRN2", target_bir_lowering=False, debug=False,
                   num_devices=N_CORES if collective else 1)
    qt_in = None
    if not collective:
        qt_in = nc.declare_dram_parameter("qtin", [DCN, DCW, B], f16,
                                          isOutput=False)

    kt_in = [nc.declare_dram_parameter(f"kt{c}", [DCW, W], f16, isOutput=False)
             for c in range(DCN)]
    et_in = nc.declare_dram_parameter("et", [DCN, DCW, TOK], f16,
                                      isOutput=False)
    wt_in = {k: nc.declare_dram_parameter(f"wt{k}", [DCW, k * DCN * KNP], f16,
                                          isOutput=False) for k in KSIZES}
    bias_in = {k: nc.declare_dram_parameter(f"bias{k}", [KNP, 1], f32,
                                            isOutput=False) for k in KSIZES}
    pm_in = [nc.declare_dram_parameter(f"pm{c}", [DCW, B], f32, isOutput=False)
             for c in range(DCN)] if balanced else None
    y_in = nc.declare_dram_parameter("yv", [B, 1], f32, isOutput=False)
    blab_in = nc.declare_dram_parameter("blab", [1, NBLK], f16, isOutput=False)

    pos_out = nc.declare_dram_parameter("pos", [B, 1], f32, isOutput=True)
    neg_out = nc.declare_dram_parameter("neg", [B, 1], f32, isOutput=True)
    ss_out = nc.declare_dram_parameter("ss", [1, BPC], f32, isOutput=True)

    cc_in = nc.dram_tensor("cc_in", [B, DCN * BPC], f16)
    cc_out = nc.dram_tensor("cc_out", [N_CORES, B, DCN * BPC], f16,
                            addr_space="Shared")

    with tile.TileContext(nc) as tc, ExitStack() as ctx:
        singles = ctx.enter_context(tc.tile_pool(name="singles", bufs=1))
        ktp = ctx.enter_context(tc.tile_pool(name="ktp", bufs=ktbufs))
        work = ctx.enter_context(tc.tile_pool(name="work", bufs=1))

        # ---------------- CNN phase ----------------
        et = []
        for dc in range(DCN):
            t = singles.tile([DCW, TOK], f16, name=f"et{dc}", tag=f"et{dc}")
            nc.sync.dma_start(out=t, in_=et_in[dc, :, :])
            et.append(t)
        wt = {}
        bia = {}
        for k in KSIZES:
            wt[k] = singles.tile([DCW, k * DCN * KNP], f16, name=f"wt{k}", tag=f"wt{k}")
            nc.sync.dma_start(out=wt[k], in_=wt_in[k][:, :])
            bia[k] = singles.tile([KNP, 1], f32, name=f"bias{k}", tag=f"bias{k}")
            nc.sync.dma_start(out=bia[k], in_=bias_in[k][:, :])

        feats = {}  # per kernel size: [100, BPC] f32 (this IS qT d-chunk)
        with tc.tile_pool(name="cnnps", bufs=2, space="PSUM") as cnnps, \
             tc.tile_pool(name="cnnsb", bufs=2) as cnnsb:
          if skip_cnn:
            ss_sb = singles.tile([1, BPC], f32, tag="ss_sb")
            nc.vector.memset(ss_sb, 0.0)
            nc.sync.dma_start(out=ss_out[:, :], in_=ss_sb[:])
          else:
              for k in KSIZES:
                  lout = L - k + 1
                  half = BPC // 2
                  fk = singles.tile([KNP, BPC], f32, name=f"feats{k}", tag=f"feats{k}")
                  feats[k] = fk
                  for h in range(2):
                      ps = cnnps.tile([KNP, half * lout], f32, tag="cnnpsum")
                      first = True
                      for t in range(k):
                          for dc in range(DCN):
                              rhs = et[dc].rearrange(
                                  "p (b l) -> p b l", l=L)[:, h * half:(h + 1) * half,
                                                           t:t + lout]
                              nc.tensor.matmul(
                                  ps[:],
                                  wt[k][:, (t * DCN + dc) * KNP:(t * DCN + dc + 1) * KNP],
                                  rhs,
                                  start=first, stop=(t == k - 1 and dc == DCN - 1))
                              first = False
                      # bias + relu (ACT), then maxpool over positions (DVE)
                      rk = cnnsb.tile([KNP, half * lout], f32, tag="relu")
                      nc.scalar.activation(rk[:], ps[:],
                                           mybir.ActivationFunctionType.Relu,
                                           bias=bia[k][:], scale=1.0)
                      nc.vector.tensor_reduce(
                          out=fk[:, h * half:(h + 1) * half],
                          in_=rk.rearrange("p (b l) -> p b l", l=lout),
                          axis=mybir.AxisListType.X, op=mybir.AluOpType.max)

              # perm-scatter local features across all 128 partitions,
              # then AllGather f16 already in the stream-lhsT layout: the
              # post-collective chain is just one readback DMA per d-chunk
              if collective:
                  fall = singles.tile([B, DCN * BPC], f16, tag="fall")
                  if balanced:
                      for i, k in enumerate(KSIZES):
                          pm = singles.tile([DCW, B], f32, name=f"pm{i}",
                                            tag=f"pm{i}")
                          nc.sync.dma_start(out=pm, in_=pm_in[i][:, :])
                          pq = cnnps.tile([B, BPC], f32, tag="pq")
                          nc.tensor.matmul(pq[:], pm[:], feats[k][:DCW, :],
                                           start=True, stop=True)
                          nc.vector.tensor_copy(
                              fall[:, i * BPC:(i + 1) * BPC], pq[:])
                  else:
                      nc.vector.memset(fall, 0.0)
                      for i, k in enumerate(KSIZES):
                          nc.vector.tensor_copy(
                              fall[:DCW, i * BPC:(i + 1) * BPC],
                              feats[k][:DCW, :])
                  nc.sync.dma_start(out=cc_in[:, :], in_=fall[:])
                  nc.gpsimd.collective_compute(
                      "AllGather", mybir.AluOpType.bypass,
                      replica_groups=[list(range(N_CORES))],
                      ins=[cc_in[:, :]], outs=[cc_out[:, :, :]])

              # sumsq of features per local batch row: ss[1, BPC]
              ones = singles.tile([DCW, 1], f32, tag="ones")
              nc.vector.memset(ones, 1.0)
              ssps = cnnps.tile([1, BPC], f32, tag="ssps")
              for i, k in enumerate(KSIZES):
                  sq = cnnsb.tile([DCW, BPC], f32, tag="sq")
                  nc.vector.tensor_mul(sq[:], feats[k][:DCW, :], feats[k][:DCW, :])
                  nc.tensor.matmul(ssps[:], ones[:], sq[:],
                                   start=(i == 0), stop=(i == len(KSIZES) - 1))
              ss_sb = singles.tile([1, BPC], f32, tag="ss_sb")
              nc.vector.tensor_copy(ss_sb[:], ssps[:])
              nc.sync.dma_start(out=ss_out[:, :], in_=ss_sb[:])

        # ---------------- AllGather readback ----------------
        qt = []
        if collective:
            for dc in range(DCN):
                qs = singles.tile([B, N_CORES, BPC], f16,
                                  name=f"qts{dc}", tag=f"qts{dc}")
                src = bass.AP(tensor=cc_out.ap().tensor,
                              offset=dc * BPC,
                              ap=[[DCN * BPC, B], [B * DCN * BPC, N_CORES],
                                  [1, BPC]])
                nc.sync.dma_start(out=qs, in_=src)
                full = qs.rearrange("p a b -> p (a b)")
                qt.append(full if balanced else full[:DCW, :])
        else:
            with tc.tile_pool(name="qperm", bufs=2, space="PSUM") as qpp:
                for dc in range(DCN):
                    q16 = singles.tile([DCW, B], f16, name=f"q16_{dc}",
                                       tag=f"q16_{dc}")
                    nc.sync.dma_start(out=q16, in_=qt_in[dc, :, :])
                    if balanced:
                        qf = singles.tile([DCW, B], f32, name=f"qf{dc}",
                                          tag=f"qf{dc}")
                        nc.vector.tensor_copy(qf[:], q16[:])
                        pm = singles.tile([DCW, B], f32, name=f"pmq{dc}",
                                          tag=f"pmq{dc}")
                        nc.sync.dma_start(out=pm, in_=pm_in[dc][:, :])
                        pq = qpp.tile([B, B], f32, tag="pq2")
                        nc.tensor.matmul(pq[:], pm[:], qf[:],
                                         start=True, stop=True)
                        qs = singles.tile([B, B], f16, name=f"qts{dc}",
                                          tag=f"qts{dc}")
                        nc.vector.tensor_copy(qs[:], pq[:])
                        qt.append(qs)
                    else:
                        qt.append(q16)

        # ---------------- mask prep (early; independent of stream) -------
        blab_b = work.tile([B, NBLK], f16, tag="blab_b")
        nc.sync.dma_start(out=blab_b, in_=bass.AP(
            tensor=blab_in.ap().tensor, offset=0, ap=[[0, B], [1, NBLK]]))
        y0 = singles.tile([B, 1], f32, tag="y0")
        nc.sync.dma_start(out=y0, in_=y_in[:, :])
        yv = singles.tile([B, 1], f32, tag="yv")
        nc.vector.tensor_copy(yv[:], y0[:])
        eq01 = work.tile([B, NBLK], f16, tag="eq01")
        nc.vector.tensor_scalar(out=eq01[:], in0=blab_b[:], scalar1=yv[:],
                                scalar2=None, op0=mybir.AluOpType.is_equal)
        # eqp = +BIG where label==y else -BIG ; eqn = the opposite
        eqp = work.tile([B, NBLK], f16, tag="eqp")
        nc.vector.tensor_scalar(out=eqp[:], in0=eq01[:], scalar1=2.0 * BIG,
                                scalar2=-BIG, op0=mybir.AluOpType.mult,
                                op1=mybir.AluOpType.add)
        eqn = work.tile([B, NBLK], f16, tag="eqn")
        nc.vector.tensor_scalar(out=eqn[:], in0=eq01[:], scalar1=-2.0 * BIG,
                                scalar2=BIG, op0=mybir.AluOpType.mult,
                                op1=mybir.AluOpType.add)

        # ---------------- memory stream ----------------
        bmall = work.tile([B, NBLK], f16, tag="bmall")
        with tc.tile_pool(name="simps", bufs=8, space="PSUM") as simps:
            # tapered tail: finer last groups so the final chunks' data
            # lands progressively earlier, shrinking the post-DMA PE tail
            sizes = []
            left = NCH
            while left > g:
                sizes.append(g)
                left -= g
            while left > 2:
                h2 = max(2, left - (left + 1) // 2)
                sizes.append((left + 1) // 2)
                left -= sizes[-1]
            if left:
                sizes.append(left)
            starts = [sum(sizes[:i]) for i in range(len(sizes))]
            for gi, (j0, gsz) in enumerate(zip(starts, sizes)):
                gw = gsz * CHUNK
                kt = []
                for dc in range(DCN):
                    if balanced:
                        t = ktp.tile([B, g * CHUNK], f16, name=f"ktt{dc}",
                                     tag=f"kt{dc}")
                        if gi < ktbufs:
                            nc.vector.memset(t, 0.0)
                        src = kt_in[dc][:, j0 * CHUNK:j0 * CHUNK + gw]
                        for i in range(3):  # stripe rows r%3==i -> p=4a+i
                            nc.sync.dma_start(out=t[i:i + 125:4, :gw],
                                              in_=src[i:96:3, :])
                        x = XTRA[dc]
                        nc.sync.dma_start(out=t[x:x + 30:8, :gw],
                                          in_=src[96:100, :])
                    else:
                        t = ktp.tile([DCW, g * CHUNK], f16, name=f"ktt{dc}",
                                     tag=f"kt{dc}")
                        nc.sync.dma_start(
                            out=t[:, :gw],
                            in_=kt_in[dc][:, j0 * CHUNK:j0 * CHUNK + gw])
                    kt.append(t)
                pss = []
                for j in range(gw // CHUNK):
                    pss.append(simps.tile([B, CHUNK], f32, name="simpsum", tag="simpsum"))
                for dc in range(DCN):
                    for j in range(gw // CHUNK):
                        nc.tensor.matmul(
                            pss[j][:], qt[dc][:],
                            kt[dc][:, j * CHUNK:(j + 1) * CHUNK],
                            start=(dc == 0), stop=(dc == DCN - 1))
                for j in range(gw // CHUNK):
                    sc = ktp.tile([B, CHUNK], f16, name="simf16", tag="simf16")
                    nc.scalar.copy(sc[:], pss[j][:])
                    nc.vector.tensor_reduce(
                        out=bmall[:, (j0 + j) * (CHUNK // BLK):
                                  (j0 + j + 1) * (CHUNK // BLK)],
                        in_=sc.rearrange("p (nb blk) -> p nb blk", blk=BLK),
                        axis=mybir.AxisListType.X, op=mybir.AluOpType.max)

        # ---------------- masked maxes over block labels ----------------
        # quarter-sliced so the early quarters overlap the tail of the
        # stream (their bmall ranges are complete mid-stream)
        NQ = 4
        QW = NBLK // NQ
        posm = work.tile([B, NBLK], f16, tag="posm")
        negm = work.tile([B, NBLK], f16, tag="negm")
        pos4 = singles.tile([B, NQ], f32, tag="pos4")
        neg4 = singles.tile([B, NQ], f32, tag="neg4")
        for q in range(NQ):
            sl = slice(q * QW, (q + 1) * QW)
            nc.vector.tensor_tensor(out=posm[:, sl], in0=bmall[:, sl],
                                    in1=eqp[:, sl], op=mybir.AluOpType.min)
            nc.vector.tensor_reduce(out=pos4[:, q:q + 1], in_=posm[:, sl],
                                    axis=mybir.AxisListType.X,
                                    op=mybir.AluOpType.max)
            nc.vector.tensor_tensor(out=negm[:, sl], in0=bmall[:, sl],
                                    in1=eqn[:, sl], op=mybir.AluOpType.min)
            nc.vector.tensor_reduce(out=neg4[:, q:q + 1], in_=negm[:, sl],
                                    axis=mybir.AxisListType.X,
                                    op=mybir.AluOpType.max)

        pos = singles.tile([B, 1], f32, tag="pos")
        nc.vector.tensor_reduce(out=pos[:], in_=pos4[:],
                                axis=mybir.AxisListType.X,
                                op=mybir.AluOpType.max)
        nc.sync.dma_start(out=pos_out[:, :], in_=pos[:])
        neg = singles.tile([B, 1], f32, tag="neg")
        nc.vector.tensor_reduce(out=neg[:], in_=neg4[:],
                                axis=mybir.AxisListType.X,
                                op=mybir.AluOpType.max)
        nc.sync.dma_start(out=neg_out[:, :], in_=neg[:])

    nc.compile()
    return nc


def _prep(x, y, embed, conv_w3, conv_b3, conv_w4, conv_b4, conv_w5, conv_b5,
          mem_keys, mem_values):
    """Host-side sharding/packing. Returns per-core input maps + combine data."""
    x = np.asarray(x)
    y64 = np.asarray(y).astype(np.int64)
    mv = np.asarray(mem_values).astype(np.int64)
    mk = np.asarray(mem_keys, dtype=np.float32)

    # --- label-sorted, block-pure padded permutation of the memory bank ---
    order = np.argsort(mv, kind="stable")
    cnt = np.bincount(mv, minlength=C)
    assert cnt.min() > 0, "kernel assumes every class present in memory"
    starts = np.zeros(C + 1, np.int64)
    starts[1:] = np.cumsum(cnt)
    parts = []
    for c in range(C):
        g = order[starts[c]:starts[c + 1]]
        padn = (-len(g)) % BLK
        if padn:
            g = np.concatenate([g, np.repeat(g[0], padn)])
        parts.append(g)
    perm = np.concatenate(parts)
    assert len(perm) <= CAP, f"padded size {len(perm)} exceeds CAP {CAP}"
    perm = np.concatenate([perm, np.repeat(perm[0], CAP - len(perm))])
    labP = mv[perm]
    blab = labP[::BLK].astype(np.float16)          # [CAP // BLK]
    keysP = mk.astype(np.float16)[perm]            # cast before gather: half the traffic

    # --- embedding lookup (host gather; device gets ready eT slabs) ---
    emb16 = np.asarray(embed, dtype=np.float32).astype(np.float16)
    e = emb16[x]                                    # [B, L, 300]
    # eT[dc, p, b*L + l] = e[b, l, dc*100 + p]
    eT = np.ascontiguousarray(
        e.reshape(B, L, DCN, DCW).transpose(2, 3, 0, 1).reshape(DCN, DCW, B * L))

    # --- conv weights: wt[k][p, (t*3+dc)*KN + kn] = w_k[kn, dc*100+p, t] ---
    wts, biases = {}, {}
    for k, w_, b_ in ((3, conv_w3, conv_b3), (4, conv_w4, conv_b4),
                      (5, conv_w5, conv_b5)):
        w_ = np.asarray(w_, dtype=np.float32)       # [KN, D, k]
        a = w_.reshape(KN, DCN, DCW, k).transpose(3, 1, 2, 0)  # [t, dc, p, kn]
        a = a.transpose(2, 0, 1, 3)                 # [p, t, dc, kn]
        ap = np.zeros((DCW, k, DCN, KNP), np.float32)
        ap[:, :, :, :KN] = a
        wts[k] = np.ascontiguousarray(
            ap.reshape(DCW, k * DCN * KNP)).astype(np.float16)
        bp = np.zeros((KNP, 1), np.float32)
        bp[:KN, 0] = np.asarray(b_, dtype=np.float32)
        biases[k] = bp

    yv = y64.astype(np.float32).reshape(B, 1)

    # permutation matrices for the port-balanced partition scatter
    pms = []
    for dc in range(DCN):
        pm = np.zeros((DCW, B), np.float32)
        for r, p in enumerate(_scatter_partitions(dc)):
            pm[r, p] = 1.0
        pms.append(pm)

    in_maps = []
    for c in range(N_CORES):
        m = {
            "et": np.ascontiguousarray(
                eT.reshape(DCN, DCW, B, L)[:, :, c * BPC:(c + 1) * BPC, :]
                .reshape(DCN, DCW, TOK)),
            "yv": yv,
            "blab": np.ascontiguousarray(
                blab[c * NBLK:(c + 1) * NBLK]).reshape(1, NBLK),
        }
        for dc in range(DCN):
            m[f"kt{dc}"] = np.ascontiguousarray(
                keysP[c * W:(c + 1) * W, dc * DCW:(dc + 1) * DCW].T)
            m[f"pm{dc}"] = pms[dc]
        for k in KSIZES:
            m[f"wt{k}"] = wts[k]
            m[f"bias{k}"] = biases[k]
        in_maps.append(m)
    return in_maps, y64


def _combine(results, y64):
    pos = np.max([r["pos"].reshape(B) for r in results], axis=0)
    neg = np.max([r["neg"].reshape(B) for r in results], axis=0)
    ss = np.concatenate([r["ss"].reshape(BPC) for r in results])  # [B]
    rn = 1.0 / np.maximum(np.sqrt(ss), 1e-12)
    sp = pos * rn
    sn = neg * rn
    loss = np.float32(np.mean(np.maximum(sn - sp + MARGIN, 0.0)))
    acc = np.float32(np.mean((sp > sn).astype(np.float32)))
    return loss, acc


def kernel(**inputs):
    global _CACHED_NC
    in_maps, y64 = _prep(**inputs)
    if _CACHED_NC is None:
        _CACHED_NC = build()
    res = run_bass_kernel_spmd(_CACHED_NC, in_maps,
                               core_ids=list(range(N_CORES)))
    return _combine(res.results, y64)



# revision 11
# speedup vs baseline: 2.4192x; 2.4192x over previous
"""Trainium2 Bass kernel for nn_CNN_Mem (CNN text encoder + cosine memory lookup).

Strategy (8 NeuronCores, SPMD):
  - Memory bank sharded along mem_size: host label-sorts mem_keys so every
    16-column block holds a single label (groups padded by duplicating a real
    key of the same label -> maxes are exact), scales by 16 and casts to
    fp8e4m3 (scale keeps all values in the fp8 normal range, so the result
    is robust to subnormal flush), then packs each core's [300, M/8] slab in
    the DoubleRow-interleaved layout: ktA[p, t, j] = K[j, t*128+p] (256 dims)
    and ktB[p, t, j] = K[j, 256+t*22+p] (44 dims).
  - CNN runs in fp8 too (embeddings and conv weights scaled by 8; the
    feature scale cancels through the norm): conv = PSUM-accumulated
    DoubleRow matmuls over shifted windows (2 matmuls per tap instead of 3
    f16 ones at twice the rate), relu+bias on ACT, maxpool on DVE.
    Features AllGather as f16, readback assembles the DoubleRow lhsT
    [128, 2, B] + [22, 2, B], cast to fp8 on DVE.
  - Stream: per 512-col chunk two DoubleRow matmuls (256-dim + 44-dim as
    22+22) accumulate fp8 sims into f32 PSUM at 0.5 cycles/row. PSUM is
    consumed in [128, 2048] tiles: ACT copies to f16 SBUF, DVE computes the
    16-wide block max with a 4-level pairwise TT-max tree (2x DVE mode).
    No memsets anywhere: every operand partition is DMA- or compute-written.
  - Label-masked maxes (sim_pos / sim_neg partials) run on DVE in quarters
    interleaved with the stream so only the last quarter trails the final
    DMA. Host combines: max over cores, normalize by feature norms (via a
    sumsq output), loss/accuracy as in the reference.

Numerics: fp8 keys + fp8 features give rel err ~7e-3 on the loss (vs 2e-2
budget) and exact accuracy; verified against an f64 host model including
fp8 subnormal flush. The fp8 DoubleRow layout and host e4m3 byte encoding
were validated bit-exactly on hardware probes.

Cost-model shape: kt DMA ~28us is the floor; ACT copies ~31us and DVE
tree+masks ~29us pace just behind it; PE ~19us incl. CNN.
"""
import numpy as np
import ml_dtypes
from contextlib import ExitStack

import concourse.bass as bass
import concourse.tile as tile
from concourse import bacc, mybir
from concourse.bass_utils import run_bass_kernel_spmd

# ---- problem dims (hardcoded; harness passes matching inputs) ----
B, L = 128, 64
V, D = 25000, 300
C = 1000
KN = 100
KSIZES = (3, 4, 5)
M, KEY = 262144, 300
MARGIN = 0.1

N_CORES = 8
BPC = B // N_CORES          # batch rows per core
TOK = BPC * L               # tokens per core
KNP = 128                   # conv output channels padded for FWL
CHUNK = 512                 # sim columns per PSUM-chunk matmul
BLK = 16                    # label-pure block width
NCH = 66                    # chunks per core
W = NCH * CHUNK             # slab columns per core (33792)
CAP = N_CORES * W           # padded memory size (270336)
NBLK = W // BLK             # blocks per core (2112)
BIG = 16384.0               # mask sentinel; |sims| <= ~8*16*64 so +-16384 acts as inf
KSCALE = 16.0               # key quantization prescale (fp8 subnormal safety)
ESCALE = 8.0                # embedding prescale
WSCALE = 8.0                # conv weight prescale

DA, DB = 256, 44            # contraction split: DoubleRow 128+128, then 22+22
PB = DB // 2                # 22

f32 = mybir.dt.float32
f16 = mybir.dt.float16
f8 = mybir.dt.float8e4
DR = mybir.MatmulPerfMode.DoubleRow
e4m3 = ml_dtypes.float8_e4m3

# stream tiling: chunks per DMA group / PSUM tile (in chunks)
GROUPS = [16] * 4 + [2]     # 66 chunks
TILES_PER_GROUP = [[2] * 8] * 4 + [[1, 1]]

_CACHED_NC = None


def build(collective=True):
    nc = bacc.Bacc("TRN2", target_bir_lowering=False, debug=False,
                   num_devices=N_CORES if collective else 1)
    qtA_in = qtB_in = None
    if not collective:
        qtA_in = nc.declare_dram_parameter("qtA", [128, 2, N_CORES, BPC], f16,
                                           isOutput=False)
        qtB_in = nc.declare_dram_parameter("qtB", [PB, 2, N_CORES, BPC], f16,
                                           isOutput=False)

    ktA_in = nc.declare_dram_parameter("ktA", [128, 2, W], f8, isOutput=False)
    ktB_in = nc.declare_dram_parameter("ktB", [PB, 2, W], f8, isOutput=False)
    NH1 = 2 * TOK + 2 * 3 * KNP
    NH2 = 2 * (4 + 5) * KNP
    hA1_in = nc.declare_dram_parameter("hA1", [128, NH1], f8, isOutput=False)
    hB1_in = nc.declare_dram_parameter("hB1", [PB, NH1], f8, isOutput=False)
    hA2_in = nc.declare_dram_parameter("hA2", [128, NH2], f8, isOutput=False)
    hB2_in = nc.declare_dram_parameter("hB2", [PB, NH2], f8, isOutput=False)
    bias_in = nc.declare_dram_parameter("biasP", [KNP, len(KSIZES)], f32,
                                        isOutput=False)
    eqp_in = nc.declare_dram_parameter("eqp", [B, NBLK], f16, isOutput=False)

    pn_out = nc.declare_dram_parameter("pn", [B, 2], f32, isOutput=True)
    ss_out = nc.declare_dram_parameter("ss", [1, BPC], f32, isOutput=True)

    if collective:
        cc_in = nc.dram_tensor("cc_in", [KN, 3 * BPC], f16)
        cc_out = nc.dram_tensor("cc_out", [N_CORES, KN, 3 * BPC], f16,
                                addr_space="Shared")

    with tile.TileContext(nc) as tc, ExitStack() as ctx:
        singles = ctx.enter_context(tc.tile_pool(name="singles", bufs=1))
        ktp = ctx.enter_context(tc.tile_pool(name="ktp", bufs=2))
        cfp = ctx.enter_context(tc.tile_pool(name="cfp", bufs=6))
        trp = ctx.enter_context(tc.tile_pool(name="trp", bufs=6))
        work = ctx.enter_context(tc.tile_pool(name="work", bufs=1))

        # ---------------- head DMAs (SP queue, packed) ----------------
        hA1 = singles.tile([128, NH1], f8, tag="hA1")
        nc.sync.dma_start(out=hA1, in_=hA1_in[:, :])
        hB1 = singles.tile([PB, NH1], f8, tag="hB1")
        nc.sync.dma_start(out=hB1, in_=hB1_in[:, :])
        biasP = singles.tile([KNP, len(KSIZES)], f32, tag="biasP")
        nc.sync.dma_start(out=biasP, in_=bias_in[:, :])
        hA2 = singles.tile([128, NH2], f8, tag="hA2")
        nc.sync.dma_start(out=hA2, in_=hA2_in[:, :])
        hB2 = singles.tile([PB, NH2], f8, tag="hB2")
        nc.sync.dma_start(out=hB2, in_=hB2_in[:, :])
        etA = hA1[:, 0:2 * TOK].rearrange("p (t x) -> p t x", t=2)
        etB = hB1[:, 0:2 * TOK].rearrange("p (t x) -> p t x", t=2)
        wtA, wtB, bia = {}, {}, {}
        for ki, k in enumerate(KSIZES):
            bia[k] = biasP[:, ki:ki + 1]
        wtA[3] = hA1[:, 2 * TOK:].rearrange("p (t x) -> p t x", t=2)
        wtB[3] = hB1[:, 2 * TOK:].rearrange("p (t x) -> p t x", t=2)
        wtA[4] = hA2[:, 0:2 * 4 * KNP].rearrange("p (t x) -> p t x", t=2)
        wtB[4] = hB2[:, 0:2 * 4 * KNP].rearrange("p (t x) -> p t x", t=2)
        wtA[5] = hA2[:, 2 * 4 * KNP:].rearrange("p (t x) -> p t x", t=2)
        wtB[5] = hB2[:, 2 * 4 * KNP:].rearrange("p (t x) -> p t x", t=2)

        if not collective:
            qt16A_e = singles.tile([128, 2, N_CORES, BPC], f16, tag="qt16Ae")
            nc.scalar.dma_start(out=qt16A_e, in_=qtA_in[:, :, :, :])
            qt16B_e = singles.tile([PB, 2, N_CORES, BPC], f16, tag="qt16Be")
            nc.scalar.dma_start(out=qt16B_e, in_=qtB_in[:, :, :, :])

        # ---------------- CNN (fp8 DoubleRow convs) ----------------
        eAv = etA.rearrange("p t (b l) -> p t b l", l=L)
        eBv = etB.rearrange("p t (b l) -> p t b l", l=L)
        feats = {}
        with tc.tile_pool(name="cnnps", bufs=3, space="PSUM") as cnnps, \
             tc.tile_pool(name="cnnsb", bufs=3) as cnnsb:
            half = BPC // 2
            fall = singles.tile([KN, 3 * BPC], f16, tag="fall")
            for ki, k in enumerate(KSIZES):
                lout = L - k + 1
                fk = singles.tile([KNP, BPC], f32, name=f"feats{k}", tag=f"feats{k}")
                feats[k] = fk
                for h in range(2):
                    ps = cnnps.tile([KNP, half * lout], f32, tag="cnnpsum")
                    for t in range(k):
                        rhsA = eAv[:, :, h * half:(h + 1) * half, t:t + lout]
                        nc.tensor.matmul(
                            ps[:], wtA[k][:, :, t * KNP:(t + 1) * KNP], rhsA,
                            start=(t == 0), stop=False, perf_mode=DR)
                        rhsB = eBv[:, :, h * half:(h + 1) * half, t:t + lout]
                        nc.tensor.matmul(
                            ps[:], wtB[k][:, :, t * KNP:(t + 1) * KNP], rhsB,
                            start=False, stop=(t == k - 1), perf_mode=DR)
                    rk = cnnsb.tile([KNP, half * lout], f32, tag="relu")
                    nc.scalar.activation(rk[:], ps[:],
                                         mybir.ActivationFunctionType.Relu,
                                         bias=bia[k][:], scale=1.0)
                    nc.vector.tensor_reduce(
                        out=fk[:, h * half:(h + 1) * half],
                        in_=rk.rearrange("p (b l) -> p b l", l=lout),
                        axis=mybir.AxisListType.X, op=mybir.AluOpType.max)
                nc.vector.tensor_copy(fall[:, ki * BPC:(ki + 1) * BPC],
                                      feats[k][:KN, :])
            if collective:
                qt16A = singles.tile([128, 2, N_CORES, BPC], f16, tag="qt16A")
                qt16B = singles.tile([PB, 2, N_CORES, BPC], f16, tag="qt16B")
                nc.scalar.dma_start(out=cc_in[:, :], in_=fall[:])
                nc.gpsimd.collective_compute(
                    "AllGather", mybir.AluOpType.bypass,
                    replica_groups=[list(range(N_CORES))],
                    ins=[cc_in[:, :]], outs=[cc_out[:, :, :]])
                # piece-wise readback into DoubleRow lhsT layout:
                # q[i=(c,b), d] with d = kidx*100 + kn; dst partition p:
                #   A t=0: d = p       ; A t=1: d = 128 + p
                #   B t=0: d = 256 + p ; B t=1: d = 278 + p
                CORE_STRIDE = KN * 3 * BPC
                pieces = [
                    (qt16A, 0, 0, 100, 0, 0),     # (dst,t,p0,np,kidx,kn0)
                    (qt16A, 0, 100, 28, 1, 0),
                    (qt16A, 1, 0, 72, 1, 28),
                    (qt16A, 1, 72, 56, 2, 0),
                    (qt16B, 0, 0, PB, 2, 56),
                    (qt16B, 1, 0, PB, 2, 78),
                ]
                for dst, t, p0, npn, kidx, kn0 in pieces:
                    src = bass.AP(
                        tensor=cc_out.ap().tensor,
                        offset=kn0 * 3 * BPC + kidx * BPC,
                        ap=[[3 * BPC, npn], [CORE_STRIDE, N_CORES], [1, BPC]])
                    nc.scalar.dma_start(out=dst[p0:p0 + npn, t, :, :], in_=src)
            else:
                qt16A, qt16B = qt16A_e, qt16B_e
            qtA8 = singles.tile([128, 2, B], f8, tag="qtA8")
            nc.vector.tensor_copy(qtA8[:], qt16A.rearrange("p t c b -> p t (c b)"))
            qtB8 = singles.tile([PB, 2, B], f8, tag="qtB8")
            nc.vector.tensor_copy(qtB8[:], qt16B.rearrange("p t c b -> p t (c b)"))

            # sumsq of local features -> ss[1, BPC]
            ones = singles.tile([KN, 1], f32, tag="ones")
            nc.gpsimd.memset(ones, 1.0)
            ssps = cnnps.tile([1, BPC], f32, tag="ssps")
            for i, k in enumerate(KSIZES):
                sq = cnnsb.tile([KN, BPC], f32, tag="sq")
                nc.vector.tensor_mul(sq[:], feats[k][:KN, :], feats[k][:KN, :])
                nc.tensor.matmul(ssps[:], ones[:], sq[:],
                                 start=(i == 0), stop=(i == len(KSIZES) - 1))
            ss_sb = singles.tile([1, BPC], f32, tag="ss_sb")
            nc.vector.tensor_copy(ss_sb[:], ssps[:])
            nc.scalar.dma_start(out=ss_out[:, :], in_=ss_sb[:])

        # ---------------- masks (host-precomputed eqp; eqn = -eqp) ----
        eqp = work.tile([B, NBLK], f16, tag="eqp")
        nc.scalar.dma_start(out=eqp, in_=eqp_in[:, :])
        eqn = work.tile([B, NBLK], f16, tag="eqn")

        # ---------------- memory stream ----------------
        bmall = work.tile([B, NBLK], f16, tag="bmall")
        NQ = 8
        QW = NBLK // NQ
        pos4 = singles.tile([B, NQ], f32, tag="pos4")
        neg4 = singles.tile([B, NQ], f32, tag="neg4")

        nc.vector.tensor_scalar(out=eqn[:], in0=eqp[:], scalar1=-1.0,
                                scalar2=None, op0=mybir.AluOpType.mult)

        def masked_quarter(q):
            sl = slice(q * QW, (q + 1) * QW)
            posm = work.tile([B, QW], f16, tag="posm")
            nc.vector.tensor_tensor(out=posm[:], in0=bmall[:, sl],
                                    in1=eqp[:, sl], op=mybir.AluOpType.min)
            nc.vector.tensor_reduce(out=pos4[:, q:q + 1], in_=posm[:],
                                    axis=mybir.AxisListType.X,
                                    op=mybir.AluOpType.max)
            negm = work.tile([B, QW], f16, tag="negm")
            nc.vector.tensor_tensor(out=negm[:], in0=bmall[:, sl],
                                    in1=eqn[:, sl], op=mybir.AluOpType.min)
            nc.vector.tensor_reduce(out=neg4[:, q:q + 1], in_=negm[:],
                                    axis=mybir.AxisListType.X,
                                    op=mybir.AluOpType.max)

        with tc.tile_pool(name="simps", bufs=4, space="PSUM") as simps:
            j0 = 0          # chunk cursor
            blk0 = 0        # block cursor (bmall columns)
            qdone = 0
            for gi, (gsz, tiles) in enumerate(zip(GROUPS, TILES_PER_GROUP)):
                gw = gsz * CHUNK
                ktA = ktp.tile([128, 2, GROUPS[0] * CHUNK], f8, tag="ktA")
                nc.sync.dma_start(out=ktA[:, :, :gw],
                                  in_=ktA_in[:, :, j0 * CHUNK:j0 * CHUNK + gw])
                ktB = ktp.tile([PB, 2, GROUPS[0] * CHUNK], f8, tag="ktB")
                nc.sync.dma_start(out=ktB[:, :, :gw],
                                  in_=ktB_in[:, :, j0 * CHUNK:j0 * CHUNK + gw])
                goff = 0
                for tsz in tiles:
                    tw = tsz * CHUNK
                    ps = simps.tile([B, TILES_PER_GROUP[0][0] * CHUNK], f32,
                                    tag="simpsum")
                    for c in range(tsz):
                        csl = slice(goff + c * CHUNK, goff + (c + 1) * CHUNK)
                        psl = ps[:, c * CHUNK:(c + 1) * CHUNK]
                        nc.tensor.matmul(psl, qtA8[:], ktA[:, :, csl],
                                         start=True, stop=False, perf_mode=DR)
                        nc.tensor.matmul(psl, qtB8[:], ktB[:, :, csl],
                                         start=False, stop=True, perf_mode=DR)
                    # ACT: PSUM f32 -> SBUF f16; DVE: 4-level pairwise max tree
                    nb = tw // BLK
                    cf = cfp.tile([B, TILES_PER_GROUP[0][0] * CHUNK], f16,
                                  tag="cf")
                    nc.scalar.copy(cf[:, :tw], ps[:, :tw])
                    v = cf[:, :tw].rearrange("p (nb blk) -> p nb blk", blk=BLK)
                    NBT = TILES_PER_GROUP[0][0] * CHUNK // BLK
                    t1 = trp.tile([B, NBT, 8], f16, tag="t1")
                    nc.vector.tensor_tensor(out=t1[:, :nb, :], in0=v[:, :, 0:8],
                                            in1=v[:, :, 8:16],
                                            op=mybir.AluOpType.max)
                    t2 = trp.tile([B, NBT, 4], f16, tag="t2")
                    nc.vector.tensor_tensor(out=t2[:, :nb, :],
                                            in0=t1[:, :nb, 0:4],
                                            in1=t1[:, :nb, 4:8],
                                            op=mybir.AluOpType.max)
                    t3 = trp.tile([B, NBT, 2], f16, tag="t3")
                    nc.vector.tensor_tensor(out=t3[:, :nb, :],
                                            in0=t2[:, :nb, 0:2],
                                            in1=t2[:, :nb, 2:4],
                                            op=mybir.AluOpType.max)
                    nc.vector.tensor_tensor(
                        out=bmall[:, blk0:blk0 + nb],
                        in0=t3[:, :nb, 0:1].rearrange("p a b -> p (a b)"),
                        in1=t3[:, :nb, 1:2].rearrange("p a b -> p (a b)"),
                        op=mybir.AluOpType.max)
                    blk0 += nb
                    goff += tw
                    # interleave completed mask quarters with the stream
                    while qdone < NQ and blk0 >= (qdone + 1) * QW:
                        masked_quarter(qdone)
                        qdone += 1
                j0 += gsz

        while qdone < NQ:
            masked_quarter(qdone)
            qdone += 1

        pn = singles.tile([B, 2], f32, tag="pn")
        nc.vector.tensor_reduce(out=pn[:, 0:1], in_=pos4[:],
                                axis=mybir.AxisListType.X,
                                op=mybir.AluOpType.max)
        nc.vector.tensor_reduce(out=pn[:, 1:2], in_=neg4[:],
                                axis=mybir.AxisListType.X,
                                op=mybir.AluOpType.max)
        nc.scalar.dma_start(out=pn_out[:, :], in_=pn[:])

    nc.compile()
    return nc


def _interleave(mat, pa):
    """[rows, 2*pa] -> [pa, 2, rows]: out[p, t, j] = mat[j, t*pa + p]."""
    return np.ascontiguousarray(
        mat.T.reshape(2, pa, -1).transpose(1, 0, 2))


def _prep(x, y, embed, conv_w3, conv_b3, conv_w4, conv_b4, conv_w5, conv_b5,
          mem_keys, mem_values):
    """Host-side sharding/packing. Returns per-core input maps."""
    x = np.asarray(x)
    y64 = np.asarray(y).astype(np.int64)
    mv = np.asarray(mem_values).astype(np.int64)
    mk = np.asarray(mem_keys, dtype=np.float32)

    # --- label-sorted, block-pure padded permutation of the memory bank ---
    order = np.argsort(mv, kind="stable")
    cnt = np.bincount(mv, minlength=C)
    assert cnt.min() > 0, "kernel assumes every class present in memory"
    starts = np.zeros(C + 1, np.int64)
    starts[1:] = np.cumsum(cnt)
    parts = []
    for c in range(C):
        g = order[starts[c]:starts[c + 1]]
        padn = (-len(g)) % BLK
        if padn:
            g = np.concatenate([g, np.repeat(g[0], padn)])
        parts.append(g)
    perm = np.concatenate(parts)
    assert len(perm) <= CAP, f"padded size {len(perm)} exceeds CAP {CAP}"
    perm = np.concatenate([perm, np.repeat(perm[0], CAP - len(perm))])
    labP = mv[perm]
    blab = labP[::BLK]                                      # [CAP // BLK]
    k8 = ((mk * KSCALE).astype(np.float16)[perm]).astype(e4m3)  # [CAP, 300]

    # --- embedding lookup, x8 scale, fp8, DoubleRow-interleave ---
    emb16 = np.asarray(embed, dtype=np.float32).astype(np.float16)
    e8 = ((emb16[x].astype(np.float32)) * ESCALE).astype(np.float16).astype(e4m3)
    # [B, L, 300] -> per-core [300, TOK]
    eT = e8.reshape(B, L, D).transpose(2, 0, 1)             # [300, B, L]

    # --- conv weights x8, fp8, interleaved [*, 2, k*KNP] ---
    wtsA, wtsB, biases = {}, {}, {}
    for k, w_, b_ in ((3, conv_w3, conv_b3), (4, conv_w4, conv_b4),
                      (5, conv_w5, conv_b5)):
        w8 = ((np.asarray(w_, dtype=np.float32)) * WSCALE).astype(
            np.float16).astype(e4m3)                        # [KN, 300, k]
        wp = np.zeros((KNP, D, k), e4m3)
        wp[:KN] = w8
        a = wp.transpose(1, 2, 0).reshape(D, k * KNP)       # [300, k*KNP]
        wtsA[k] = _interleave(a.T[:, :DA].reshape(k * KNP, DA), 128)
        wtsB[k] = _interleave(a.T[:, DA:].reshape(k * KNP, DB), PB)
        bp = np.zeros((KNP, 1), np.float32)
        bp[:KN, 0] = np.asarray(b_, dtype=np.float32) * (ESCALE * WSCALE)
        biases[k] = bp

    eqp_full = np.where(blab[None, :] == y64[:, None], BIG, -BIG).astype(
        np.float16)                                         # [B, CAP//BLK]

    in_maps = []
    for c in range(N_CORES):
        sl = k8[c * W:(c + 1) * W]                          # [W, 300]
        eloc = eT[:, c * BPC:(c + 1) * BPC, :].reshape(D, TOK)
        m = {
            "ktA": _interleave(sl[:, :DA], 128),
            "ktB": _interleave(sl[:, DA:], PB),
            "etA": _interleave(eloc.T[:, :DA], 128),
            "etB": _interleave(eloc.T[:, DA:], PB),
            "eqp": np.ascontiguousarray(eqp_full[:, c * NBLK:(c + 1) * NBLK]),
        }
        m["hA1"] = np.concatenate(
            [m.pop("etA").reshape(128, -1), wtsA[3].reshape(128, -1)], axis=1)
        m["hB1"] = np.concatenate(
            [m.pop("etB").reshape(PB, -1), wtsB[3].reshape(PB, -1)], axis=1)
        m["hA2"] = np.concatenate(
            [wtsA[4].reshape(128, -1), wtsA[5].reshape(128, -1)], axis=1)
        m["hB2"] = np.concatenate(
            [wtsB[4].reshape(PB, -1), wtsB[5].reshape(PB, -1)], axis=1)
        m["biasP"] = np.concatenate([biases[k] for k in KSIZES], axis=1)
        in_maps.append(m)
    return in_maps, y64


def _combine(results, y64):
    pos = np.max([r["pn"][:, 0] for r in results], axis=0)
    neg = np.max([r["pn"][:, 1] for r in results], axis=0)
    ss = np.concatenate([r["ss"].reshape(BPC) for r in results])  # [B]
    rn = 1.0 / np.maximum(np.sqrt(ss), 1e-12)
    sp = pos * rn / KSCALE
    sn = neg * rn / KSCALE
    loss = np.float32(np.mean(np.maximum(sn - sp + MARGIN, 0.0)))
    acc = np.float32(np.mean((sp > sn).astype(np.float32)))
    return loss, acc


def kernel(**inputs):
    global _CACHED_NC
    in_maps, y64 = _prep(**inputs)
    if _CACHED_NC is None:
        _CACHED_NC = build()
    res = run_bass_kernel_spmd(_CACHED_NC, in_maps,
                               core_ids=list(range(N_CORES)))
    return _combine(res.results, y64)


# revision 23
# speedup vs baseline: 2.5226x; 1.0427x over previous
"""Trainium2 Bass kernel for nn_CNN_Mem (CNN text encoder + cosine memory lookup).

Strategy (8 NeuronCores, SPMD):
  - Memory bank sharded along mem_size: host label-sorts mem_keys so every
    16-column block holds a single label (groups padded by duplicating a real
    key of the same label -> maxes are exact), scales by 16 and casts to
    fp8e4m3 (scale keeps all values in the fp8 normal range, so the result
    is robust to subnormal flush), then packs each core's [300, M/8] slab in
    the DoubleRow-interleaved layout: ktA[p, t, j] = K[j, t*128+p] (256 dims)
    and ktB[p, t, j] = K[j, 256+t*22+p] (44 dims).
  - CNN runs in fp8 too (embeddings and conv weights scaled by 8; the
    feature scale cancels through the norm): conv = PSUM-accumulated
    DoubleRow matmuls over shifted windows (2 matmuls per tap instead of 3
    f16 ones at twice the rate), relu+bias on ACT, maxpool on DVE.
    Features AllGather as f16, readback assembles the DoubleRow lhsT
    [128, 2, B] + [22, 2, B], cast to fp8 on DVE.
  - Stream: per 512-col chunk two DoubleRow matmuls (256-dim + 44-dim as
    22+22) accumulate fp8 sims into f32 PSUM at 0.5 cycles/row. PSUM is
    consumed in [128, 2048] tiles: ACT copies to f16 SBUF, DVE computes the
    16-wide block max with a 4-level pairwise TT-max tree (2x DVE mode).
    No memsets anywhere: every operand partition is DMA- or compute-written.
  - Label-masked maxes (sim_pos / sim_neg partials) run on DVE in quarters
    interleaved with the stream so only the last quarter trails the final
    DMA. Host combines: max over cores, normalize by feature norms (via a
    sumsq output), loss/accuracy as in the reference.

Numerics: fp8 keys + fp8 features give rel err ~7e-3 on the loss (vs 2e-2
budget) and exact accuracy; verified against an f64 host model including
fp8 subnormal flush. The fp8 DoubleRow layout and host e4m3 byte encoding
were validated bit-exactly on hardware probes.

Cost-model shape: kt DMA ~28us is the floor; ACT copies ~31us and DVE
tree+masks ~29us pace just behind it; PE ~19us incl. CNN.
"""
import numpy as np
import ml_dtypes
from contextlib import ExitStack

import concourse.bass as bass
import concourse.tile as tile
from concourse import bacc, mybir
from concourse.bass_utils import run_bass_kernel_spmd

# ---- problem dims (hardcoded; harness passes matching inputs) ----
B, L = 128, 64
V, D = 25000, 300
C = 1000
KN = 100
KSIZES = (3, 4, 5)
M, KEY = 262144, 300
MARGIN = 0.1

N_CORES = 8
BPC = B // N_CORES          # batch rows per core
TOK = BPC * L               # tokens per core
KNP = 128                   # conv output channels padded for FWL
CHUNK = 512                 # sim columns per PSUM-chunk matmul
BLK = 16                    # label-pure block width
NCH = 66                    # chunks per core
W = NCH * CHUNK             # slab columns per core (33792)
CAP = N_CORES * W           # padded memory size (270336)
NBLK = W // BLK             # blocks per core (2112)
BIG = 16384.0               # mask sentinel; |sims| <= ~8*16*64 so +-16384 acts as inf
KSCALE = 16.0               # key quantization prescale (fp8 subnormal safety)
ESCALE = 8.0                # embedding prescale
WSCALE = 8.0                # conv weight prescale

DA, DB = 256, 44            # contraction split: DoubleRow 128+128, then 22+22
PB = DB // 2                # 22

f32 = mybir.dt.float32
f16 = mybir.dt.float16
f8 = mybir.dt.float8e4
DR = mybir.MatmulPerfMode.DoubleRow
e4m3 = ml_dtypes.float8_e4m3

# stream tiling: chunks per DMA group / PSUM tile (in chunks)
GROUPS = [8, 8, 16, 16, 16, 2]     # 66 chunks
TILES_PER_GROUP = [[2] * 4, [2] * 4, [2] * 8, [2] * 8, [2] * 8, [1, 1]]

_CACHED_NC = None


def build(collective=True):
    nc = bacc.Bacc("TRN2", target_bir_lowering=False, debug=False,
                   num_devices=N_CORES if collective else 1)
    qtA_in = qtB_in = None
    if not collective:
        qtA_in = nc.declare_dram_parameter("qtA", [128, 2, N_CORES, BPC], f16,
                                           isOutput=False)
        qtB_in = nc.declare_dram_parameter("qtB", [PB, 2, N_CORES, BPC], f16,
                                           isOutput=False)

    ktA_in = nc.declare_dram_parameter("ktA", [128, 2, W], f8, isOutput=False)
    ktB_in = nc.declare_dram_parameter("ktB", [PB, 2, W], f8, isOutput=False)
    NH1 = 2 * TOK + 2 * 3 * KNP
    NH2 = 2 * (4 + 5) * KNP
    hA1_in = nc.declare_dram_parameter("hA1", [128, NH1], f8, isOutput=False)
    hB1_in = nc.declare_dram_parameter("hB1", [PB, NH1], f8, isOutput=False)
    hA2_in = nc.declare_dram_parameter("hA2", [128, NH2], f8, isOutput=False)
    hB2_in = nc.declare_dram_parameter("hB2", [PB, NH2], f8, isOutput=False)
    bias_in = nc.declare_dram_parameter("biasP", [KNP, len(KSIZES)], f32,
                                        isOutput=False)
    eqp_in = nc.declare_dram_parameter("eqp", [B, NBLK], f16, isOutput=False)

    pn_out = nc.declare_dram_parameter("pn", [B, 2], f32, isOutput=True)
    ss_out = nc.declare_dram_parameter("ss", [1, BPC], f32, isOutput=True)

    if collective:
        cc_in = nc.dram_tensor("cc_in", [KN, 3 * BPC], f16)
        cc_out = nc.dram_tensor("cc_out", [N_CORES, KN, 3 * BPC], f16,
                                addr_space="Shared")

    with tile.TileContext(nc) as tc, ExitStack() as ctx:
        singles = ctx.enter_context(tc.tile_pool(name="singles", bufs=1))
        ktp = ctx.enter_context(tc.tile_pool(name="ktp", bufs=3))
        cfp = ctx.enter_context(tc.tile_pool(name="cfp", bufs=8))
        trp = ctx.enter_context(tc.tile_pool(name="trp", bufs=8))
        work = ctx.enter_context(tc.tile_pool(name="work", bufs=1))

        # ---------------- head DMAs (SP queue, packed) ----------------
        hA1 = singles.tile([128, NH1], f8, tag="hA1")
        nc.sync.dma_start(out=hA1, in_=hA1_in[:, :])
        hB1 = singles.tile([PB, NH1], f8, tag="hB1")
        nc.sync.dma_start(out=hB1, in_=hB1_in[:, :])
        biasP = singles.tile([KNP, len(KSIZES)], f32, tag="biasP")
        nc.sync.dma_start(out=biasP, in_=bias_in[:, :])
        hA2 = singles.tile([128, NH2], f8, tag="hA2")
        nc.sync.dma_start(out=hA2, in_=hA2_in[:, :])
        hB2 = singles.tile([PB, NH2], f8, tag="hB2")
        nc.sync.dma_start(out=hB2, in_=hB2_in[:, :])
        etA = hA1[:, 0:2 * TOK].rearrange("p (t x) -> p t x", t=2)
        etB = hB1[:, 0:2 * TOK].rearrange("p (t x) -> p t x", t=2)
        wtA, wtB, bia = {}, {}, {}
        for ki, k in enumerate(KSIZES):
            bia[k] = biasP[:, ki:ki + 1]
        wtA[3] = hA1[:, 2 * TOK:].rearrange("p (t x) -> p t x", t=2)
        wtB[3] = hB1[:, 2 * TOK:].rearrange("p (t x) -> p t x", t=2)
        wtA[4] = hA2[:, 0:2 * 4 * KNP].rearrange("p (t x) -> p t x", t=2)
        wtB[4] = hB2[:, 0:2 * 4 * KNP].rearrange("p (t x) -> p t x", t=2)
        wtA[5] = hA2[:, 2 * 4 * KNP:].rearrange("p (t x) -> p t x", t=2)
        wtB[5] = hB2[:, 2 * 4 * KNP:].rearrange("p (t x) -> p t x", t=2)

        if not collective:
            qt16A_e = singles.tile([128, 2, N_CORES, BPC], f16, tag="qt16Ae")
            nc.scalar.dma_start(out=qt16A_e, in_=qtA_in[:, :, :, :])
            qt16B_e = singles.tile([PB, 2, N_CORES, BPC], f16, tag="qt16Be")
            nc.scalar.dma_start(out=qt16B_e, in_=qtB_in[:, :, :, :])

        # ---------------- CNN (fp8 DoubleRow convs) ----------------
        eAv = etA.rearrange("p t (b l) -> p t b l", l=L)
        eBv = etB.rearrange("p t (b l) -> p t b l", l=L)
        feats = {}
        with tc.tile_pool(name="cnnps", bufs=3, space="PSUM") as cnnps, \
             tc.tile_pool(name="cnnsb", bufs=3) as cnnsb:
            half = BPC // 2
            fall = singles.tile([KN, 3 * BPC], f16, tag="fall")
            for ki, k in enumerate(KSIZES):
                lout = L - k + 1
                fk = singles.tile([KNP, BPC], f32, name=f"feats{k}", tag=f"feats{k}")
                feats[k] = fk
                for h in range(2):
                    ps = cnnps.tile([KNP, half * lout], f32, tag="cnnpsum")
                    for t in range(k):
                        rhsA = eAv[:, :, h * half:(h + 1) * half, t:t + lout]
                        nc.tensor.matmul(
                            ps[:], wtA[k][:, :, t * KNP:(t + 1) * KNP], rhsA,
                            start=(t == 0), stop=False, perf_mode=DR)
                        rhsB = eBv[:, :, h * half:(h + 1) * half, t:t + lout]
                        nc.tensor.matmul(
                            ps[:], wtB[k][:, :, t * KNP:(t + 1) * KNP], rhsB,
                            start=False, stop=(t == k - 1), perf_mode=DR)
                    rk = cnnsb.tile([KNP, half * lout], f32, tag="relu")
                    nc.scalar.activation(rk[:], ps[:],
                                         mybir.ActivationFunctionType.Relu,
                                         bias=bia[k][:], scale=1.0)
                    nc.vector.tensor_reduce(
                        out=fk[:, h * half:(h + 1) * half],
                        in_=rk.rearrange("p (b l) -> p b l", l=lout),
                        axis=mybir.AxisListType.X, op=mybir.AluOpType.max)
                nc.vector.tensor_copy(fall[:, ki * BPC:(ki + 1) * BPC],
                                      feats[k][:KN, :])
            if collective:
                qt16A = singles.tile([128, 2, N_CORES, BPC], f16, tag="qt16A")
                qt16B = singles.tile([PB, 2, N_CORES, BPC], f16, tag="qt16B")
                nc.scalar.dma_start(out=cc_in[:, :], in_=fall[:])
                nc.gpsimd.collective_compute(
                    "AllGather", mybir.AluOpType.bypass,
                    replica_groups=[list(range(N_CORES))],
                    ins=[cc_in[:, :]], outs=[cc_out[:, :, :]])
                # piece-wise readback into DoubleRow lhsT layout:
                # q[i=(c,b), d] with d = kidx*100 + kn; dst partition p:
                #   A t=0: d = p       ; A t=1: d = 128 + p
                #   B t=0: d = 256 + p ; B t=1: d = 278 + p
                CORE_STRIDE = KN * 3 * BPC
                pieces = [
                    (qt16A, 0, 0, 100, 0, 0),     # (dst,t,p0,np,kidx,kn0)
                    (qt16A, 0, 100, 28, 1, 0),
                    (qt16A, 1, 0, 72, 1, 28),
                    (qt16A, 1, 72, 56, 2, 0),
                    (qt16B, 0, 0, PB, 2, 56),
                    (qt16B, 1, 0, PB, 2, 78),
                ]
                for dst, t, p0, npn, kidx, kn0 in pieces:
                    src = bass.AP(
                        tensor=cc_out.ap().tensor,
                        offset=kn0 * 3 * BPC + kidx * BPC,
                        ap=[[3 * BPC, npn], [CORE_STRIDE, N_CORES], [1, BPC]])
                    nc.scalar.dma_start(out=dst[p0:p0 + npn, t, :, :], in_=src)
            else:
                qt16A, qt16B = qt16A_e, qt16B_e
            qtA8 = singles.tile([128, 2, B], f8, tag="qtA8")
            nc.vector.tensor_copy(qtA8[:], qt16A.rearrange("p t c b -> p t (c b)"))
            qtB8 = singles.tile([PB, 2, B], f8, tag="qtB8")
            nc.vector.tensor_copy(qtB8[:], qt16B.rearrange("p t c b -> p t (c b)"))

            # sumsq of local features -> ss[1, BPC]
            ones = singles.tile([KN, 1], f32, tag="ones")
            nc.gpsimd.memset(ones, 1.0)
            ssps = cnnps.tile([1, BPC], f32, tag="ssps")
            for i, k in enumerate(KSIZES):
                sq = cnnsb.tile([KN, BPC], f32, tag="sq")
                nc.vector.tensor_mul(sq[:], feats[k][:KN, :], feats[k][:KN, :])
                nc.tensor.matmul(ssps[:], ones[:], sq[:],
                                 start=(i == 0), stop=(i == len(KSIZES) - 1))
            ss_sb = singles.tile([1, BPC], f32, tag="ss_sb")
            nc.vector.tensor_copy(ss_sb[:], ssps[:])
            nc.scalar.dma_start(out=ss_out[:, :], in_=ss_sb[:])

        # ---------------- masks (host-precomputed eqp; eqn = -eqp) ----
        eqp = work.tile([B, NBLK], f16, tag="eqp")
        nc.sync.dma_start(out=eqp, in_=eqp_in[:, :])
        eqn = work.tile([B, NBLK], f16, tag="eqn")

        # ---------------- memory stream ----------------
        bmall = work.tile([B, NBLK], f16, tag="bmall")
        NQ = 8
        QW = NBLK // NQ
        pos4 = singles.tile([B, NQ], f32, tag="pos4")
        neg4 = singles.tile([B, NQ], f32, tag="neg4")

        def masked_quarter(q):
            if q == 0:
                nc.vector.tensor_scalar(out=eqn[:], in0=eqp[:], scalar1=-1.0,
                                        scalar2=None, op0=mybir.AluOpType.mult)
            sl = slice(q * QW, (q + 1) * QW)
            posm = work.tile([B, QW], f16, tag="posm")
            nc.vector.tensor_tensor(out=posm[:], in0=bmall[:, sl],
                                    in1=eqp[:, sl], op=mybir.AluOpType.min)
            nc.vector.tensor_reduce(out=pos4[:, q:q + 1], in_=posm[:],
                                    axis=mybir.AxisListType.X,
                                    op=mybir.AluOpType.max)
            negm = work.tile([B, QW], f16, tag="negm")
            nc.vector.tensor_tensor(out=negm[:], in0=bmall[:, sl],
                                    in1=eqn[:, sl], op=mybir.AluOpType.min)
            nc.vector.tensor_reduce(out=neg4[:, q:q + 1], in_=negm[:],
                                    axis=mybir.AxisListType.X,
                                    op=mybir.AluOpType.max)

        with tc.tile_pool(name="simps", bufs=4, space="PSUM") as simps:
            j0 = 0          # chunk cursor
            blk0 = 0        # block cursor (bmall columns)
            qdone = 0
            tile_idx = 0
            for gi, (gsz, tiles) in enumerate(zip(GROUPS, TILES_PER_GROUP)):
                gw = gsz * CHUNK
                ktA = ktp.tile([128, 2, max(GROUPS) * CHUNK], f8, tag="ktA")
                nc.sync.dma_start(out=ktA[:, :, :gw],
                                  in_=ktA_in[:, :, j0 * CHUNK:j0 * CHUNK + gw])
                ktB = ktp.tile([PB, 2, max(GROUPS) * CHUNK], f8, tag="ktB")
                nc.sync.dma_start(out=ktB[:, :, :gw],
                                  in_=ktB_in[:, :, j0 * CHUNK:j0 * CHUNK + gw])
                goff = 0
                for tsz in tiles:
                    tw = tsz * CHUNK
                    ps = simps.tile([B, 2 * CHUNK], f32,
                                    tag="simpsum")
                    for c in range(tsz):
                        csl = slice(goff + c * CHUNK, goff + (c + 1) * CHUNK)
                        psl = ps[:, c * CHUNK:(c + 1) * CHUNK]
                        nc.tensor.matmul(psl, qtA8[:], ktA[:, :, csl],
                                         start=True, stop=False, perf_mode=DR)
                        nc.tensor.matmul(psl, qtB8[:], ktB[:, :, csl],
                                         start=False, stop=True, perf_mode=DR)
                    # ACT: PSUM f32 -> SBUF f16; DVE: 4-level pairwise max tree
                    # (every 11th tile: DVE reduces PSUM directly, no ACT copy,
                    #  to balance the two engines)
                    nb = tw // BLK
                    if tile_idx % 14 == 5:
                        nc.vector.tensor_reduce(
                            out=bmall[:, blk0:blk0 + nb],
                            in_=ps[:, :tw].rearrange(
                                "p (nb blk) -> p nb blk", blk=BLK),
                            axis=mybir.AxisListType.X, op=mybir.AluOpType.max)
                        blk0 += nb
                        goff += tw
                        tile_idx += 1
                        while qdone < NQ and blk0 >= (qdone + 1) * QW:
                            masked_quarter(qdone)
                            qdone += 1
                        continue
                    cf = cfp.tile([B, 2 * CHUNK], f16, tag="cf")
                    nc.scalar.copy(cf[:, :tw], ps[:, :tw])
                    v = cf[:, :tw].rearrange("p (nb blk) -> p nb blk", blk=BLK)
                    NBT = 2 * CHUNK // BLK
                    t1 = trp.tile([B, NBT, 8], f16, tag="t1")
                    nc.vector.tensor_tensor(out=t1[:, :nb, :], in0=v[:, :, 0:8],
                                            in1=v[:, :, 8:16],
                                            op=mybir.AluOpType.max)
                    t2 = trp.tile([B, NBT, 4], f16, tag="t2")
                    nc.vector.tensor_tensor(out=t2[:, :nb, :],
                                            in0=t1[:, :nb, 0:4],
                                            in1=t1[:, :nb, 4:8],
                                            op=mybir.AluOpType.max)
                    t3 = trp.tile([B, NBT, 2], f16, tag="t3")
                    nc.vector.tensor_tensor(out=t3[:, :nb, :],
                                            in0=t2[:, :nb, 0:2],
                                            in1=t2[:, :nb, 2:4],
                                            op=mybir.AluOpType.max)
                    nc.vector.tensor_tensor(
                        out=bmall[:, blk0:blk0 + nb],
                        in0=t3[:, :nb, 0:1].rearrange("p a b -> p (a b)"),
                        in1=t3[:, :nb, 1:2].rearrange("p a b -> p (a b)"),
                        op=mybir.AluOpType.max)
                    blk0 += nb
                    goff += tw
                    tile_idx += 1
                    # interleave completed mask quarters with the stream
                    while qdone < NQ and blk0 >= (qdone + 1) * QW:
                        masked_quarter(qdone)
                        qdone += 1
                j0 += gsz

        while qdone < NQ:
            masked_quarter(qdone)
            qdone += 1

        pn = singles.tile([B, 2], f32, tag="pn")
        nc.vector.tensor_reduce(out=pn[:, 0:1], in_=pos4[:],
                                axis=mybir.AxisListType.X,
                                op=mybir.AluOpType.max)
        nc.vector.tensor_reduce(out=pn[:, 1:2], in_=neg4[:],
                                axis=mybir.AxisListType.X,
                                op=mybir.AluOpType.max)
        nc.scalar.dma_start(out=pn_out[:, :], in_=pn[:])

    nc.compile()
    return nc


def _interleave(mat, pa):
    """[rows, 2*pa] -> [pa, 2, rows]: out[p, t, j] = mat[j, t*pa + p]."""
    return np.ascontiguousarray(
        mat.T.reshape(2, pa, -1).transpose(1, 0, 2))


def _prep(x, y, embed, conv_w3, conv_b3, conv_w4, conv_b4, conv_w5, conv_b5,
          mem_keys, mem_values):
    """Host-side sharding/packing. Returns per-core input maps."""
    x = np.asarray(x)
    y64 = np.asarray(y).astype(np.int64)
    mv = np.asarray(mem_values).astype(np.int64)
    mk = np.asarray(mem_keys, dtype=np.float32)

    # --- label-sorted, block-pure padded permutation of the memory bank ---
    order = np.argsort(mv, kind="stable")
    cnt = np.bincount(mv, minlength=C)
    assert cnt.min() > 0, "kernel assumes every class present in memory"
    starts = np.zeros(C + 1, np.int64)
    starts[1:] = np.cumsum(cnt)
    parts = []
    for c in range(C):
        g = order[starts[c]:starts[c + 1]]
        padn = (-len(g)) % BLK
        if padn:
            g = np.concatenate([g, np.repeat(g[0], padn)])
        parts.append(g)
    perm = np.concatenate(parts)
    assert len(perm) <= CAP, f"padded size {len(perm)} exceeds CAP {CAP}"
    perm = np.concatenate([perm, np.repeat(perm[0], CAP - len(perm))])
    labP = mv[perm]
    blab = labP[::BLK]                                      # [CAP // BLK]
    k8 = ((mk * KSCALE).astype(np.float16)[perm]).astype(e4m3)  # [CAP, 300]

    # --- embedding lookup, x8 scale, fp8, DoubleRow-interleave ---
    emb16 = np.asarray(embed, dtype=np.float32).astype(np.float16)
    e8 = ((emb16[x].astype(np.float32)) * ESCALE).astype(np.float16).astype(e4m3)
    # [B, L, 300] -> per-core [300, TOK]
    eT = e8.reshape(B, L, D).transpose(2, 0, 1)             # [300, B, L]

    # --- conv weights x8, fp8, interleaved [*, 2, k*KNP] ---
    wtsA, wtsB, biases = {}, {}, {}
    for k, w_, b_ in ((3, conv_w3, conv_b3), (4, conv_w4, conv_b4),
                      (5, conv_w5, conv_b5)):
        w8 = ((np.asarray(w_, dtype=np.float32)) * WSCALE).astype(
            np.float16).astype(e4m3)                        # [KN, 300, k]
        wp = np.zeros((KNP, D, k), e4m3)
        wp[:KN] = w8
        a = wp.transpose(1, 2, 0).reshape(D, k * KNP)       # [300, k*KNP]
        wtsA[k] = _interleave(a.T[:, :DA].reshape(k * KNP, DA), 128)
        wtsB[k] = _interleave(a.T[:, DA:].reshape(k * KNP, DB), PB)
        bp = np.zeros((KNP, 1), np.float32)
        bp[:KN, 0] = np.asarray(b_, dtype=np.float32) * (ESCALE * WSCALE)
        biases[k] = bp

    eqp_full = np.where(blab[None, :] == y64[:, None], BIG, -BIG).astype(
        np.float16)                                         # [B, CAP//BLK]

    in_maps = []
    for c in range(N_CORES):
        sl = k8[c * W:(c + 1) * W]                          # [W, 300]
        eloc = eT[:, c * BPC:(c + 1) * BPC, :].reshape(D, TOK)
        m = {
            "ktA": _interleave(sl[:, :DA], 128),
            "ktB": _interleave(sl[:, DA:], PB),
            "etA": _interleave(eloc.T[:, :DA], 128),
            "etB": _interleave(eloc.T[:, DA:], PB),
            "eqp": np.ascontiguousarray(eqp_full[:, c * NBLK:(c + 1) * NBLK]),
        }
        m["hA1"] = np.concatenate(
            [m.pop("etA").reshape(128, -1), wtsA[3].reshape(128, -1)], axis=1)
        m["hB1"] = np.concatenate(
            [m.pop("etB").reshape(PB, -1), wtsB[3].reshape(PB, -1)], axis=1)
        m["hA2"] = np.concatenate(
            [wtsA[4].reshape(128, -1), wtsA[5].reshape(128, -1)], axis=1)
        m["hB2"] = np.concatenate(
            [wtsB[4].reshape(PB, -1), wtsB[5].reshape(PB, -1)], axis=1)
        m["biasP"] = np.concatenate([biases[k] for k in KSIZES], axis=1)
        in_maps.append(m)
    return in_maps, y64


def _combine(results, y64):
    pos = np.max([r["pn"][:, 0] for r in results], axis=0)
    neg = np.max([r["pn"][:, 1] for r in results], axis=0)
    ss = np.concatenate([r["ss"].reshape(BPC) for r in results])  # [B]
    rn = 1.0 / np.maximum(np.sqrt(ss), 1e-12)
    sp = pos * rn / KSCALE
    sn = neg * rn / KSCALE
    loss = np.float32(np.mean(np.maximum(sn - sp + MARGIN, 0.0)))
    acc = np.float32(np.mean((sp > sn).astype(np.float32)))
    return loss, acc


def kernel(**inputs):
    global _CACHED_NC
    in_maps, y64 = _prep(**inputs)
    if _CACHED_NC is None:
        _CACHED_NC = build()
    res = run_bass_kernel_spmd(_CACHED_NC, in_maps,
                               core_ids=list(range(N_CORES)))
    return _combine(res.results, y64)


# revision 26
# speedup vs baseline: 2.5344x; 1.0047x over previous
"""Trainium2 Bass kernel for nn_CNN_Mem (CNN text encoder + cosine memory lookup).

Strategy (8 NeuronCores, SPMD):
  - Memory bank sharded along mem_size: host label-sorts mem_keys so every
    16-column block holds a single label (groups padded by duplicating a real
    key of the same label -> maxes are exact), scales by 16 and casts to
    fp8e4m3 (scale keeps all values in the fp8 normal range, so the result
    is robust to subnormal flush), then packs each core's [300, M/8] slab in
    the DoubleRow-interleaved layout: ktA[p, t, j] = K[j, t*128+p] (256 dims)
    and ktB[p, t, j] = K[j, 256+t*22+p] (44 dims).
  - CNN runs in fp8 too (embeddings and conv weights scaled by 8; the
    feature scale cancels through the norm): conv = PSUM-accumulated
    DoubleRow matmuls over shifted windows (2 matmuls per tap instead of 3
    f16 ones at twice the rate), relu+bias on ACT, maxpool on DVE.
    Features AllGather as f16, readback assembles the DoubleRow lhsT
    [128, 2, B] + [22, 2, B], cast to fp8 on DVE.
  - Stream: per 512-col chunk two DoubleRow matmuls (256-dim + 44-dim as
    22+22) accumulate fp8 sims into f32 PSUM at 0.5 cycles/row. PSUM is
    consumed in [128, 2048] tiles: ACT copies to f16 SBUF, DVE computes the
    16-wide block max with a 4-level pairwise TT-max tree (2x DVE mode).
    No memsets anywhere: every operand partition is DMA- or compute-written.
  - Label-masked maxes (sim_pos / sim_neg partials) run on DVE in quarters
    interleaved with the stream so only the last quarter trails the final
    DMA. Host combines: max over cores, normalize by feature norms (via a
    sumsq output), loss/accuracy as in the reference.

Numerics: fp8 keys + fp8 features give rel err ~7e-3 on the loss (vs 2e-2
budget) and exact accuracy; verified against an f64 host model including
fp8 subnormal flush. The fp8 DoubleRow layout and host e4m3 byte encoding
were validated bit-exactly on hardware probes.

Cost-model shape: kt DMA ~28us is the floor; ACT copies ~31us and DVE
tree+masks ~29us pace just behind it; PE ~19us incl. CNN.
"""
import numpy as np
import ml_dtypes
from contextlib import ExitStack

import concourse.bass as bass
import concourse.tile as tile
from concourse import bacc, mybir
from concourse.bass_utils import run_bass_kernel_spmd

# ---- problem dims (hardcoded; harness passes matching inputs) ----
B, L = 128, 64
V, D = 25000, 300
C = 1000
KN = 100
KSIZES = (3, 4, 5)
M, KEY = 262144, 300
MARGIN = 0.1

N_CORES = 8
BPC = B // N_CORES          # batch rows per core
TOK = BPC * L               # tokens per core
KNP = 128                   # conv output channels padded for FWL
CHUNK = 512                 # sim columns per PSUM-chunk matmul
BLK = 16                    # label-pure block width
NCH = 66                    # chunks per core
W = NCH * CHUNK             # slab columns per core (33792)
CAP = N_CORES * W           # padded memory size (270336)
NBLK = W // BLK             # blocks per core (2112)
BIG = 16384.0               # mask sentinel; |sims| <= ~8*16*64 so +-16384 acts as inf
KSCALE = 16.0               # key quantization prescale (fp8 subnormal safety)
ESCALE = 8.0                # embedding prescale
WSCALE = 8.0                # conv weight prescale

DA, DB = 256, 44            # contraction split: DoubleRow 128+128, then 22+22
PB = DB // 2                # 22

f32 = mybir.dt.float32
f16 = mybir.dt.float16
f8 = mybir.dt.float8e4
DR = mybir.MatmulPerfMode.DoubleRow
e4m3 = ml_dtypes.float8_e4m3

# stream tiling: chunks per DMA group / PSUM tile (in chunks)
GROUPS = [8, 8, 16, 16, 16, 2]     # 66 chunks
TILES_PER_GROUP = [[2] * 4, [2] * 4, [2] * 8, [2] * 8, [2] * 8, [1, 1]]

WARMUP_MM = 20

_CACHED_NC = None


def build(collective=True):
    nc = bacc.Bacc("TRN2", target_bir_lowering=False, debug=False,
                   num_devices=N_CORES if collective else 1)
    qtA_in = qtB_in = None
    if not collective:
        qtA_in = nc.declare_dram_parameter("qtA", [128, 2, N_CORES, BPC], f16,
                                           isOutput=False)
        qtB_in = nc.declare_dram_parameter("qtB", [PB, 2, N_CORES, BPC], f16,
                                           isOutput=False)

    ktA_in = nc.declare_dram_parameter("ktA", [128, 2, W], f8, isOutput=False)
    ktB_in = nc.declare_dram_parameter("ktB", [PB, 2, W], f8, isOutput=False)
    NH1 = 2 * TOK + 2 * 3 * KNP
    NH2 = 2 * (4 + 5) * KNP
    hA1_in = nc.declare_dram_parameter("hA1", [128, NH1], f8, isOutput=False)
    hB1_in = nc.declare_dram_parameter("hB1", [PB, NH1], f8, isOutput=False)
    hA2_in = nc.declare_dram_parameter("hA2", [128, NH2], f8, isOutput=False)
    hB2_in = nc.declare_dram_parameter("hB2", [PB, NH2], f8, isOutput=False)
    bias_in = nc.declare_dram_parameter("biasP", [KNP, len(KSIZES)], f32,
                                        isOutput=False)
    eqp_in = nc.declare_dram_parameter("eqp", [B, NBLK], f16, isOutput=False)

    pn_out = nc.declare_dram_parameter("pn", [B, 2], f32, isOutput=True)
    ss_out = nc.declare_dram_parameter("ss", [1, BPC], f32, isOutput=True)

    if collective:
        cc_in = nc.dram_tensor("cc_in", [KN, 3 * BPC], f16)
        cc_out = nc.dram_tensor("cc_out", [N_CORES, KN, 3 * BPC], f16,
                                addr_space="Shared")

    with tile.TileContext(nc) as tc, ExitStack() as ctx:
        singles = ctx.enter_context(tc.tile_pool(name="singles", bufs=1))
        ktp = ctx.enter_context(tc.tile_pool(name="ktp", bufs=3))
        cfp = ctx.enter_context(tc.tile_pool(name="cfp", bufs=8))
        trp = ctx.enter_context(tc.tile_pool(name="trp", bufs=8))
        work = ctx.enter_context(tc.tile_pool(name="work", bufs=1))

        # ---------------- head DMAs (SP queue, packed) ----------------
        hA1 = singles.tile([128, NH1], f8, tag="hA1")
        nc.sync.dma_start(out=hA1, in_=hA1_in[:, :])
        hB1 = singles.tile([PB, NH1], f8, tag="hB1")
        nc.sync.dma_start(out=hB1, in_=hB1_in[:, :])
        biasP = singles.tile([KNP, len(KSIZES)], f32, tag="biasP")
        nc.sync.dma_start(out=biasP, in_=bias_in[:, :])
        hA2 = singles.tile([128, NH2], f8, tag="hA2")
        nc.sync.dma_start(out=hA2, in_=hA2_in[:, :])
        hB2 = singles.tile([PB, NH2], f8, tag="hB2")
        nc.sync.dma_start(out=hB2, in_=hB2_in[:, :])
        etA = hA1[:, 0:2 * TOK].rearrange("p (t x) -> p t x", t=2)
        etB = hB1[:, 0:2 * TOK].rearrange("p (t x) -> p t x", t=2)
        wtA, wtB, bia = {}, {}, {}
        for ki, k in enumerate(KSIZES):
            bia[k] = biasP[:, ki:ki + 1]
        wtA[3] = hA1[:, 2 * TOK:].rearrange("p (t x) -> p t x", t=2)
        wtB[3] = hB1[:, 2 * TOK:].rearrange("p (t x) -> p t x", t=2)
        wtA[4] = hA2[:, 0:2 * 4 * KNP].rearrange("p (t x) -> p t x", t=2)
        wtB[4] = hB2[:, 0:2 * 4 * KNP].rearrange("p (t x) -> p t x", t=2)
        wtA[5] = hA2[:, 2 * 4 * KNP:].rearrange("p (t x) -> p t x", t=2)
        wtB[5] = hB2[:, 2 * 4 * KNP:].rearrange("p (t x) -> p t x", t=2)

        if not collective:
            qt16A_e = singles.tile([128, 2, N_CORES, BPC], f16, tag="qt16Ae")
            nc.scalar.dma_start(out=qt16A_e, in_=qtA_in[:, :, :, :])
            qt16B_e = singles.tile([PB, 2, N_CORES, BPC], f16, tag="qt16Be")
            nc.scalar.dma_start(out=qt16B_e, in_=qtB_in[:, :, :, :])

        # ---------------- PE p-state warmup during head DMAs ----------
        # dummy matmuls keep the PE continuously busy so the conv matmuls
        # start at full clock instead of mid-ramp
        wmt = singles.tile([128, 64], f16, tag="wmt")
        nc.vector.memset(wmt, 0.0)
        with tc.tile_pool(name="wmps", bufs=1, space="PSUM") as wmps:
            wps = wmps.tile([64, 64], f32, tag="wmps")
            for _ in range(WARMUP_MM):
                nc.tensor.matmul(wps[:], wmt[:, 0:64], wmt[:, 0:64],
                                 start=True, stop=True)

        # ---------------- CNN (fp8 DoubleRow convs) ----------------
        eAv = etA.rearrange("p t (b l) -> p t b l", l=L)
        eBv = etB.rearrange("p t (b l) -> p t b l", l=L)
        feats = {}
        with tc.tile_pool(name="cnnps", bufs=3, space="PSUM") as cnnps, \
             tc.tile_pool(name="cnnsb", bufs=3) as cnnsb:
            half = BPC // 2
            fall = singles.tile([KN, 3 * BPC], f16, tag="fall")
            for ki, k in enumerate(KSIZES):
                lout = L - k + 1
                fk = singles.tile([KNP, BPC], f32, name=f"feats{k}", tag=f"feats{k}")
                feats[k] = fk
                for h in range(2):
                    ps = cnnps.tile([KNP, half * lout], f32, tag="cnnpsum")
                    for t in range(k):
                        rhsA = eAv[:, :, h * half:(h + 1) * half, t:t + lout]
                        nc.tensor.matmul(
                            ps[:], wtA[k][:, :, t * KNP:(t + 1) * KNP], rhsA,
                            start=(t == 0), stop=False, perf_mode=DR)
                        rhsB = eBv[:, :, h * half:(h + 1) * half, t:t + lout]
                        nc.tensor.matmul(
                            ps[:], wtB[k][:, :, t * KNP:(t + 1) * KNP], rhsB,
                            start=False, stop=(t == k - 1), perf_mode=DR)
                    rk = cnnsb.tile([KNP, half * lout], f32, tag="relu")
                    nc.scalar.activation(rk[:], ps[:],
                                         mybir.ActivationFunctionType.Relu,
                                         bias=bia[k][:], scale=1.0)
                    nc.vector.tensor_reduce(
                        out=fk[:, h * half:(h + 1) * half],
                        in_=rk.rearrange("p (b l) -> p b l", l=lout),
                        axis=mybir.AxisListType.X, op=mybir.AluOpType.max)
                nc.vector.tensor_copy(fall[:, ki * BPC:(ki + 1) * BPC],
                                      feats[k][:KN, :])
            if collective:
                qt16A = singles.tile([128, 2, N_CORES, BPC], f16, tag="qt16A")
                qt16B = singles.tile([PB, 2, N_CORES, BPC], f16, tag="qt16B")
                nc.scalar.dma_start(out=cc_in[:, :], in_=fall[:])
                nc.gpsimd.collective_compute(
                    "AllGather", mybir.AluOpType.bypass,
                    replica_groups=[list(range(N_CORES))],
                    ins=[cc_in[:, :]], outs=[cc_out[:, :, :]])
                # piece-wise readback into DoubleRow lhsT layout:
                # q[i=(c,b), d] with d = kidx*100 + kn; dst partition p:
                #   A t=0: d = p       ; A t=1: d = 128 + p
                #   B t=0: d = 256 + p ; B t=1: d = 278 + p
                CORE_STRIDE = KN * 3 * BPC
                pieces = [
                    (qt16A, 0, 0, 100, 0, 0),     # (dst,t,p0,np,kidx,kn0)
                    (qt16A, 0, 100, 28, 1, 0),
                    (qt16A, 1, 0, 72, 1, 28),
                    (qt16A, 1, 72, 56, 2, 0),
                    (qt16B, 0, 0, PB, 2, 56),
                    (qt16B, 1, 0, PB, 2, 78),
                ]
                for dst, t, p0, npn, kidx, kn0 in pieces:
                    src = bass.AP(
                        tensor=cc_out.ap().tensor,
                        offset=kn0 * 3 * BPC + kidx * BPC,
                        ap=[[3 * BPC, npn], [CORE_STRIDE, N_CORES], [1, BPC]])
                    nc.scalar.dma_start(out=dst[p0:p0 + npn, t, :, :], in_=src)
            else:
                qt16A, qt16B = qt16A_e, qt16B_e
            qtA8 = singles.tile([128, 2, B], f8, tag="qtA8")
            nc.vector.tensor_copy(qtA8[:], qt16A.rearrange("p t c b -> p t (c b)"))
            qtB8 = singles.tile([PB, 2, B], f8, tag="qtB8")
            nc.vector.tensor_copy(qtB8[:], qt16B.rearrange("p t c b -> p t (c b)"))

            # sumsq of local features -> ss[1, BPC]
            ones = singles.tile([KN, 1], f32, tag="ones")
            nc.gpsimd.memset(ones, 1.0)
            ssps = cnnps.tile([1, BPC], f32, tag="ssps")
            for i, k in enumerate(KSIZES):
                sq = cnnsb.tile([KN, BPC], f32, tag="sq")
                nc.vector.tensor_mul(sq[:], feats[k][:KN, :], feats[k][:KN, :])
                nc.tensor.matmul(ssps[:], ones[:], sq[:],
                                 start=(i == 0), stop=(i == len(KSIZES) - 1))
            ss_sb = singles.tile([1, BPC], f32, tag="ss_sb")
            nc.vector.tensor_copy(ss_sb[:], ssps[:])
            nc.scalar.dma_start(out=ss_out[:, :], in_=ss_sb[:])

        # ---------------- masks (host-precomputed eqp; eqn = -eqp) ----
        eqp = work.tile([B, NBLK], f16, tag="eqp")
        nc.sync.dma_start(out=eqp, in_=eqp_in[:, :])
        eqn = work.tile([B, NBLK], f16, tag="eqn")

        # ---------------- memory stream ----------------
        bmall = work.tile([B, NBLK], f16, tag="bmall")
        NQ = 8
        QW = NBLK // NQ
        pos4 = singles.tile([B, NQ], f32, tag="pos4")
        neg4 = singles.tile([B, NQ], f32, tag="neg4")

        def masked_quarter(q):
            if q == 0:
                nc.vector.tensor_scalar(out=eqn[:], in0=eqp[:], scalar1=-1.0,
                                        scalar2=None, op0=mybir.AluOpType.mult)
            sl = slice(q * QW, (q + 1) * QW)
            posm = work.tile([B, QW], f16, tag="posm")
            nc.vector.tensor_tensor(out=posm[:], in0=bmall[:, sl],
                                    in1=eqp[:, sl], op=mybir.AluOpType.min)
            nc.vector.tensor_reduce(out=pos4[:, q:q + 1], in_=posm[:],
                                    axis=mybir.AxisListType.X,
                                    op=mybir.AluOpType.max)
            negm = work.tile([B, QW], f16, tag="negm")
            nc.vector.tensor_tensor(out=negm[:], in0=bmall[:, sl],
                                    in1=eqn[:, sl], op=mybir.AluOpType.min)
            nc.vector.tensor_reduce(out=neg4[:, q:q + 1], in_=negm[:],
                                    axis=mybir.AxisListType.X,
                                    op=mybir.AluOpType.max)

        with tc.tile_pool(name="simps", bufs=4, space="PSUM") as simps:
            j0 = 0          # chunk cursor
            blk0 = 0        # block cursor (bmall columns)
            qdone = 0
            tile_idx = 0
            for gi, (gsz, tiles) in enumerate(zip(GROUPS, TILES_PER_GROUP)):
                gw = gsz * CHUNK
                ktA = ktp.tile([128, 2, max(GROUPS) * CHUNK], f8, tag="ktA")
                nc.sync.dma_start(out=ktA[:, :, :gw],
                                  in_=ktA_in[:, :, j0 * CHUNK:j0 * CHUNK + gw])
                ktB = ktp.tile([PB, 2, max(GROUPS) * CHUNK], f8, tag="ktB")
                nc.sync.dma_start(out=ktB[:, :, :gw],
                                  in_=ktB_in[:, :, j0 * CHUNK:j0 * CHUNK + gw])
                goff = 0
                for tsz in tiles:
                    tw = tsz * CHUNK
                    ps = simps.tile([B, 2 * CHUNK], f32,
                                    tag="simpsum")
                    for c in range(tsz):
                        csl = slice(goff + c * CHUNK, goff + (c + 1) * CHUNK)
                        psl = ps[:, c * CHUNK:(c + 1) * CHUNK]
                        nc.tensor.matmul(psl, qtA8[:], ktA[:, :, csl],
                                         start=True, stop=False, perf_mode=DR)
                        nc.tensor.matmul(psl, qtB8[:], ktB[:, :, csl],
                                         start=False, stop=True, perf_mode=DR)
                    # ACT: PSUM f32 -> SBUF f16; DVE: 4-level pairwise max tree
                    # (every 11th tile: DVE reduces PSUM directly, no ACT copy,
                    #  to balance the two engines)
                    nb = tw // BLK
                    if tile_idx % 14 == 5:
                        nc.vector.tensor_reduce(
                            out=bmall[:, blk0:blk0 + nb],
                            in_=ps[:, :tw].rearrange(
                                "p (nb blk) -> p nb blk", blk=BLK),
                            axis=mybir.AxisListType.X, op=mybir.AluOpType.max)
                        blk0 += nb
                        goff += tw
                        tile_idx += 1
                        while qdone < NQ and blk0 >= (qdone + 1) * QW:
                            masked_quarter(qdone)
                            qdone += 1
                        continue
                    cf = cfp.tile([B, 2 * CHUNK], f16, tag="cf")
                    nc.scalar.copy(cf[:, :tw], ps[:, :tw])
                    v = cf[:, :tw].rearrange("p (nb blk) -> p nb blk", blk=BLK)
                    NBT = 2 * CHUNK // BLK
                    t1 = trp.tile([B, NBT, 8], f16, tag="t1")
                    nc.vector.tensor_tensor(out=t1[:, :nb, :], in0=v[:, :, 0:8],
                                            in1=v[:, :, 8:16],
                                            op=mybir.AluOpType.max)
                    t2 = trp.tile([B, NBT, 4], f16, tag="t2")
                    nc.vector.tensor_tensor(out=t2[:, :nb, :],
                                            in0=t1[:, :nb, 0:4],
                                            in1=t1[:, :nb, 4:8],
                                            op=mybir.AluOpType.max)
                    t3 = trp.tile([B, NBT, 2], f16, tag="t3")
                    nc.vector.tensor_tensor(out=t3[:, :nb, :],
                                            in0=t2[:, :nb, 0:2],
                                            in1=t2[:, :nb, 2:4],
                                            op=mybir.AluOpType.max)
                    nc.vector.tensor_tensor(
                        out=bmall[:, blk0:blk0 + nb],
                        in0=t3[:, :nb, 0:1].rearrange("p a b -> p (a b)"),
                        in1=t3[:, :nb, 1:2].rearrange("p a b -> p (a b)"),
                        op=mybir.AluOpType.max)
                    blk0 += nb
                    goff += tw
                    tile_idx += 1
                    # interleave completed mask quarters with the stream
                    while qdone < NQ and blk0 >= (qdone + 1) * QW:
                        masked_quarter(qdone)
                        qdone += 1
                j0 += gsz

        while qdone < NQ:
            masked_quarter(qdone)
            qdone += 1

        pn = singles.tile([B, 2], f32, tag="pn")
        nc.vector.tensor_reduce(out=pn[:, 0:1], in_=pos4[:],
                                axis=mybir.AxisListType.X,
                                op=mybir.AluOpType.max)
        nc.vector.tensor_reduce(out=pn[:, 1:2], in_=neg4[:],
                                axis=mybir.AxisListType.X,
                                op=mybir.AluOpType.max)
        nc.scalar.dma_start(out=pn_out[:, :], in_=pn[:])

    nc.compile()
    return nc


def _interleave(mat, pa):
    """[rows, 2*pa] -> [pa, 2, rows]: out[p, t, j] = mat[j, t*pa + p]."""
    return np.ascontiguousarray(
        mat.T.reshape(2, pa, -1).transpose(1, 0, 2))


def _prep(x, y, embed, conv_w3, conv_b3, conv_w4, conv_b4, conv_w5, conv_b5,
          mem_keys, mem_values):
    """Host-side sharding/packing. Returns per-core input maps."""
    x = np.asarray(x)
    y64 = np.asarray(y).astype(np.int64)
    mv = np.asarray(mem_values).astype(np.int64)
    mk = np.asarray(mem_keys, dtype=np.float32)

    # --- label-sorted, block-pure padded permutation of the memory bank ---
    order = np.argsort(mv, kind="stable")
    cnt = np.bincount(mv, minlength=C)
    assert cnt.min() > 0, "kernel assumes every class present in memory"
    starts = np.zeros(C + 1, np.int64)
    starts[1:] = np.cumsum(cnt)
    parts = []
    for c in range(C):
        g = order[starts[c]:starts[c + 1]]
        padn = (-len(g)) % BLK
        if padn:
            g = np.concatenate([g, np.repeat(g[0], padn)])
        parts.append(g)
    perm = np.concatenate(parts)
    assert len(perm) <= CAP, f"padded size {len(perm)} exceeds CAP {CAP}"
    perm = np.concatenate([perm, np.repeat(perm[0], CAP - len(perm))])
    labP = mv[perm]
    blab = labP[::BLK]                                      # [CAP // BLK]
    k8 = ((mk * KSCALE).astype(np.float16)[perm]).astype(e4m3)  # [CAP, 300]

    # --- embedding lookup, x8 scale, fp8, DoubleRow-interleave ---
    emb16 = np.asarray(embed, dtype=np.float32).astype(np.float16)
    e8 = ((emb16[x].astype(np.float32)) * ESCALE).astype(np.float16).astype(e4m3)
    # [B, L, 300] -> per-core [300, TOK]
    eT = e8.reshape(B, L, D).transpose(2, 0, 1)             # [300, B, L]

    # --- conv weights x8, fp8, interleaved [*, 2, k*KNP] ---
    wtsA, wtsB, biases = {}, {}, {}
    for k, w_, b_ in ((3, conv_w3, conv_b3), (4, conv_w4, conv_b4),
                      (5, conv_w5, conv_b5)):
        w8 = ((np.asarray(w_, dtype=np.float32)) * WSCALE).astype(
            np.float16).astype(e4m3)                        # [KN, 300, k]
        wp = np.zeros((KNP, D, k), e4m3)
        wp[:KN] = w8
        a = wp.transpose(1, 2, 0).reshape(D, k * KNP)       # [300, k*KNP]
        wtsA[k] = _interleave(a.T[:, :DA].reshape(k * KNP, DA), 128)
        wtsB[k] = _interleave(a.T[:, DA:].reshape(k * KNP, DB), PB)
        bp = np.zeros((KNP, 1), np.float32)
        bp[:KN, 0] = np.asarray(b_, dtype=np.float32) * (ESCALE * WSCALE)
        biases[k] = bp

    eqp_full = np.where(blab[None, :] == y64[:, None], BIG, -BIG).astype(
        np.float16)                                         # [B, CAP//BLK]

    in_maps = []
    for c in range(N_CORES):
        sl = k8[c * W:(c + 1) * W]                          # [W, 300]
        eloc = eT[:, c * BPC:(c + 1) * BPC, :].reshape(D, TOK)
        m = {
            "ktA": _interleave(sl[:, :DA], 128),
            "ktB": _interleave(sl[:, DA:], PB),
            "etA": _interleave(eloc.T[:, :DA], 128),
            "etB": _interleave(eloc.T[:, DA:], PB),
            "eqp": np.ascontiguousarray(eqp_full[:, c * NBLK:(c + 1) * NBLK]),
        }
        m["hA1"] = np.concatenate(
            [m.pop("etA").reshape(128, -1), wtsA[3].reshape(128, -1)], axis=1)
        m["hB1"] = np.concatenate(
            [m.pop("etB").reshape(PB, -1), wtsB[3].reshape(PB, -1)], axis=1)
        m["hA2"] = np.concatenate(
            [wtsA[4].reshape(128, -1), wtsA[5].reshape(128, -1)], axis=1)
        m["hB2"] = np.concatenate(
            [wtsB[4].reshape(PB, -1), wtsB[5].reshape(PB, -1)], axis=1)
        m["biasP"] = np.concatenate([biases[k] for k in KSIZES], axis=1)
        in_maps.append(m)
    return in_maps, y64


def _combine(results, y64):
    pos = np.max([r["pn"][:, 0] for r in results], axis=0)
    neg = np.max([r["pn"][:, 1] for r in results], axis=0)
    ss = np.concatenate([r["ss"].reshape(BPC) for r in results])  # [B]
    rn = 1.0 / np.maximum(np.sqrt(ss), 1e-12)
    sp = pos * rn / KSCALE
    sn = neg * rn / KSCALE
    loss = np.float32(np.mean(np.maximum(sn - sp + MARGIN, 0.0)))
    acc = np.float32(np.mean((sp > sn).astype(np.float32)))
    return loss, acc


def kernel(**inputs):
    global _CACHED_NC
    in_maps, y64 = _prep(**inputs)
    if _CACHED_NC is None:
        _CACHED_NC = build()
    res = run_bass_kernel_spmd(_CACHED_NC, in_maps,
                               core_ids=list(range(N_CORES)))
    return _combine(res.results, y64)
